# revision 2
# baseline (speedup 1.0000x reference)
"""Fused single-launch Trainium2 kernel for nn_AttentionLayer_46677704572976.

Algebraic reduction (see original kernel.py): the softmax is over a size-1
axis -> attention matrix is all-ones -> out[0,c,h,w] = s[(96h+w) mod 512]
with s = (colsum_fold(conv_features) + colsum(pe)) @ wV.T.

Single SPMD launch on 8 cores:
  1. stream x shard (bf16, [128, 4608] layout) + ag shard (f32) + weights
  2. fold mod-512 + channel-reduce via PE matmuls -> S_loc (1,512) f32
  3. 2 KB DRAM AllReduce of S_loc -> S_glob on every core
  4. +pe, cast bf16, XBAR DMA-transpose to [128,16] partition-major
  5. 4 matmuls (stationary = S columns broadcast, moving = rolled wV.T bf16)
     -> psum bc[128,512]: every partition holds s (rolled per core)
  6. window W[128,1152] = tile(bc) -> 4 column-block DMA writes of the
     (512, 1152) output shard in [128, 4608] layout.
"""

import sys

for _p in ("/opt/trn_rl_repo",):
    if _p not in sys.path:
        sys.path.append(_p)

import numpy as np
import ml_dtypes

import concourse.bass as bass
import concourse.tile as tile
from concourse import bacc, mybir
from concourse.bass_utils import run_bass_kernel_spmd

NCORES = 8
D = 512
IMG = 96
N = IMG * IMG
PIX = N // NCORES        # 1152
HSH = IMG // NCORES      # 12
AGPIX = 288
F32 = mybir.dt.float32
BF16 = mybir.dt.bfloat16
NPBF16 = ml_dtypes.bfloat16

_NC = None
LAST_RESULTS = None


def _build():
    nc = bacc.Bacc("TRN2", target_bir_lowering=False, debug=False,
                   num_devices=NCORES)

    xs = nc.declare_dram_parameter("xs", [128, 4608], BF16, isOutput=False)
    ags = nc.declare_dram_parameter("ags", [128, 2 * AGPIX], F32, isOutput=False)
    u1p = nc.declare_dram_parameter("u1p", [128, 4], BF16, isOutput=False)
    vvp = nc.declare_dram_parameter("vvp", [128, 8], F32, isOutput=False)
    pep = nc.declare_dram_parameter("pep", [1, D], F32, isOutput=False)
    wvb = nc.declare_dram_parameter("wvb", [128, 2048], BF16, isOutput=False)
    out = nc.declare_dram_parameter("out", [128, 4608], F32, isOutput=True)

    with tile.TileContext(nc) as tc:
        with (
            tc.tile_pool(name="sb", bufs=1) as sb,
            tc.tile_pool(name="ps", bufs=1, space="PSUM") as ps,
            tc.tile_pool(name="dram", bufs=2, space="DRAM") as dram,
        ):
            # ---- small weight loads first (3 HWDGE queues) ----
            u1s = sb.tile([128, 4], BF16)
            nc.sync.dma_start(u1s[:], u1p[:, :])
            vvs = sb.tile([128, 8], F32)
            nc.scalar.dma_start(vvs[:], vvp[:, :])
            pes = sb.tile([1, D], F32)
            nc.scalar.dma_start(pes[:], pep[:, :])
            agc = sb.tile([128, 2 * AGPIX], F32)
            nc.gpsimd.dma_start(agc[:], ags[:, :])
            AG = [agc[:, 0:AGPIX], agc[:, AGPIX:2 * AGPIX]]

            # ---- bulk x: 3 column chunks of [128, 1536] on 3 queues ----
            xsb = sb.tile([128, 4608], BF16)
            xengs = (nc.sync, nc.scalar, nc.gpsimd)
            for c in range(3):
                xengs[c].dma_start(xsb[:, 1536 * c:1536 * (c + 1)],
                                   xs[:, 1536 * c:1536 * (c + 1)])
            # rolled wV.T bf16, split over sync+scalar (overlaps x stream)
            wvs = sb.tile([128, 2048], BF16)
            nc.sync.dma_start(wvs[:, 0:1024], wvb[:, 0:1024])
            nc.scalar.dma_start(wvs[:, 1024:2048], wvb[:, 1024:2048])

            # PE clock warm-up
            wu = ps.tile([1, 4], F32)
            for _ in range(2):
                nc.tensor.matmul(wu[:1, :], u1s[:, 0:1], u1s[:, 0:4],
                                 start=True, stop=True)

            # ---- ag path (f32, same as two-launch version) ----
            AD = ps.tile([4, AGPIX], F32)
            for k in range(2):
                nc.tensor.matmul(AD[:, :], vvs[:, 4 * k:4 * k + 4], AG[k][:, :],
                                 start=(k == 0), stop=(k == 1))
            adsb = sb.tile([4, AGPIX], F32)
            nc.vector.tensor_copy(adsb[:, :], AD[:, :])
            tba = sb.tile([1, PIX], F32)
            nc.gpsimd.dma_start(tba[:1, :], adsb[:, :])
            tbp = sb.tile([1, PIX], F32)
            tb3 = tbp[:1, :].rearrange("p (y q) -> p y q", y=6)
            for a in range(2):
                for b in range(2):
                    st = 96 * a + b
                    dst = tb3[:, :, st:st + 95:2]
                    s_ = tba[:1, 288 * (2 * a + b):288 * (2 * a + b) + 288
                             ].rearrange("p (y x) -> p y x", y=6)
                    nc.vector.tensor_copy(dst, s_)
            tbf = sb.tile([1, D], F32)
            nc.vector.tensor_add(tbf[:1, :], tbp[:1, 0:512], tbp[:1, 512:1024])
            nc.vector.tensor_add(tbf[:1, 0:128], tbf[:1, 0:128],
                                 tbp[:1, 1024:1152])

            # ---- x fold: 12 matmuls, fold + channel-reduce in PSUM ----
            S = ps.tile([1, D], F32)
            S2 = ps.tile([1, 128], F32)
            for k in range(4):
                base = 1152 * k
                nc.tensor.matmul(S[:1, :], u1s[:, k:k + 1],
                                 xsb[:, base:base + 512],
                                 start=(k == 0), stop=False)
                nc.tensor.matmul(S[:1, :], u1s[:, k:k + 1],
                                 xsb[:, base + 512:base + 1024],
                                 start=False, stop=(k == 3))
                nc.tensor.matmul(S2[:1, :], u1s[:, k:k + 1],
                                 xsb[:, base + 1024:base + 1152],
                                 start=(k == 0), stop=(k == 3))

            srow = sb.tile([1, D], F32)
            nc.vector.tensor_add(srow[:1, :], S[:1, :], tbf[:1, :])
            nc.vector.tensor_add(srow[:1, 0:128], srow[:1, 0:128], S2[:1, :])

            # ---- AllGather the 8 partials, roll-fold on device ----
            # core i's local fold group t is global group (t + 128*i) % 512,
            # so partials must be rotated before summing -- AllGather all 8
            # and apply the (fixed, core-independent) rotations here.
            ib = dram.tile([1, D], F32)
            ob = dram.tile([NCORES, D], F32)
            nc.gpsimd.dma_start(ib[:], srow[:1, :])
            nc.gpsimd.collective_compute(
                "AllGather", mybir.AluOpType.bypass,
                replica_groups=[list(range(NCORES))],
                ins=[ib.opt()], outs=[ob.opt()],
            )
            srt = sb.tile([1, NCORES * D], F32)
            nc.sync.dma_start(
                srt[:1, :],
                ob[:, :].rearrange("a b -> () (a b)"))
            srow2 = sb.tile([1, D], F32)
            # i = 0 and 4 (roll 0): full-width add seeds the accumulator
            nc.vector.tensor_add(srow2[:1, :], srt[:1, 0:512],
                                 srt[:1, 4 * D:4 * D + 512])
            for i in (1, 2, 3, 5, 6, 7):
                m = i % 4
                lo = 512 * i
                w = 512 - 128 * m
                nc.vector.tensor_add(srow2[:1, 128 * m:512],
                                     srow2[:1, 128 * m:512],
                                     srt[:1, lo:lo + w])
                nc.vector.tensor_add(srow2[:1, 0:128 * m],
                                     srow2[:1, 0:128 * m],
                                     srt[:1, lo + w:lo + 512])

            # ---- +pe, cast bf16, pad, bounce, XBAR transpose ----
            spad = sb.tile([1, 2048], BF16)
            nc.vector.memset(spad[:1, 512:2048], 0.0)
            nc.vector.tensor_add(srow2[:1, :], srow2[:1, :], pes[:1, :])
            nc.vector.tensor_copy(spad[:1, 0:512], srow2[:1, :])
            b2 = dram.tile([1, 2048], BF16)
            nc.scalar.dma_start(b2[:], spad[:1, :])
            sgs = sb.tile([128, 16], BF16)
            nc.sync.dma_start(
                sgs[:, :],
                b2[0:1, :].rearrange("p (a b) -> (p a) b", a=16),
                transpose=True)

            # ---- projection: bc[p, j] = s_rolled[j] for every p ----
            bc = ps.tile([128, D], F32)
            for k in range(4):
                lhsT = sgs[:, k:k + 1].to_broadcast((128, 128))
                nc.tensor.matmul(bc[:, :], lhsT, wvs[:, 512 * k:512 * (k + 1)],
                                 start=(k == 0), stop=(k == 3))
            bss = sb.tile([128, D], F32)
            nc.vector.tensor_copy(bss[:, :], bc[:, :])

            # ---- window + 4 column-block writes on 4 queues ----
            W = sb.tile([128, PIX], F32)
            nc.vector.tensor_copy(W[:, 0:512], bss[:, :])
            nc.vector.tensor_copy(W[:, 512:1024], bss[:, :])
            nc.vector.tensor_copy(W[:, 1024:1152], bss[:, 0:128])
            wengs = (nc.sync, nc.scalar, nc.gpsimd, nc.sync)
            for k in range(4):
                wengs[k].dma_start(out[:, 1152 * k:1152 * (k + 1)], W[:, :])

    nc.compile()
    return nc


def _get_nc():
    global _NC
    if _NC is None:
        _NC = _build()
    return _NC


def _pe_colsum():
    pos = np.arange(N, dtype=np.float64)
    msk = np.arange(D)
    cos_msk = 1.0 - (msk % 2).astype(np.float64)
    freqs = (1e-4) ** ((2 * (msk // 2)).astype(np.float64) / D)
    ang = pos[:, None] * freqs[None, :]
    return (np.cos(ang) * cos_msk + np.sin(ang) * (1.0 - cos_msk)).sum(axis=0)


def kernel(x, y, ag, w_up, w_kv, w_q1, wQ, wK, wV):
    global LAST_RESULTS
    x = np.ascontiguousarray(x, dtype=np.float32)
    ag = np.ascontiguousarray(ag, dtype=np.float32)
    w_up = np.asarray(w_up, dtype=np.float32)
    w_kv = np.asarray(w_kv, dtype=np.float32)
    wV = np.ascontiguousarray(wV, dtype=np.float32)

    u = w_kv.sum(axis=0)
    u1, u2 = u[:D], u[D:]
    v = np.einsum('iokw,o->ikw', w_up, u2)

    u1p = np.ascontiguousarray(u1.reshape(4, 128).T).astype(NPBF16)
    vv = v.reshape(256, 4)
    vvp = np.ascontiguousarray(
        vv.reshape(2, 128, 4).transpose(1, 0, 2).reshape(128, 8))
    pep = _pe_colsum().astype(np.float32).reshape(1, D)

    x2 = x.reshape(D, IMG, IMG)
    ag2 = ag.reshape(256, 48, 48)

    nc = _get_nc()

    in_maps = []
    for i in range(NCORES):
        xsh = x2[:, HSH * i:HSH * (i + 1), :].reshape(D, PIX)
        xs = np.ascontiguousarray(
            xsh.reshape(4, 128, PIX).transpose(1, 0, 2).reshape(128, 4608)
        ).astype(NPBF16)
        wvt = np.roll(wV, -128 * (i % 4), axis=0).T          # (512g, 512j)
        wvb = np.ascontiguousarray(
            wvt.reshape(4, 128, 512).transpose(1, 0, 2).reshape(128, 2048)
        ).astype(NPBF16)
        in_maps.append({
            "xs": xs,
            "ags": np.ascontiguousarray(
                ag2[:, 6 * i:6 * (i + 1), :].reshape(2, 128, AGPIX)
                .transpose(1, 0, 2).reshape(128, 2 * AGPIX)),
            "u1p": u1p, "vvp": vvp, "pep": pep, "wvb": wvb,
        })
    res = run_bass_kernel_spmd(nc, in_maps, list(range(NCORES)))
    LAST_RESULTS = (res,)

    out2 = np.empty((D, N), dtype=np.float32)
    for i in range(NCORES):
        sh = res.results[i]["out"]                           # (128, 4608)
        out2[:, PIX * i:PIX * (i + 1)] = (
            sh.reshape(128, 4, PIX).transpose(1, 0, 2).reshape(D, PIX))
    return out2.reshape(1, D, IMG, IMG)


# revision 3
# speedup vs baseline: 2.4021x; 2.4021x over previous
"""Trainium2 Bass kernel for nn_AttentionLayer_46677704572976.

Key algebraic fact: the reference applies softmax over a size-1 axis, so the
attention matrix is exactly all-ones and

    out[0, c, h, w] = s[(96*h + w) mod 512]   (independent of c)
    s = (fold_512(conv_features) + colsum(pe)) @ wV.T

where fold_512 sums input-feature columns over pixel groups p mod 512, and
the conv features collapse onto per-pixel dot products with folded weights
(u = colsum(w_kv), v = w_up @ u2).  y / wQ / wK / w_q1 are dead.

Two SPMD launches over 8 cores (device collectives cost ~52us fixed in this
runtime, so the 2 KB cross-core exchange goes through the host):

  A) per-core fold of its x/ag shard -> S_loc (512 floats).  x ships as bf16
     in a [128, 4608] channel-tiled layout (partition p holds x rows
     p, p+128, p+256, p+384); fold + channel reduction happen in one PSUM
     accumulation of 12 matmuls.  ag path stays f32.
  B) host sums the rotated partials (+ pe colsum) -> s_glob; each core then
     computes s = rolled_wV @ s_glob on device (rotation folded into
     per-core weight data; s_glob rides in the same DMA as the weights) and
     broadcast-writes its (512, 1152) column shard of the output.
"""

import sys

for _p in ("/opt/trn_rl_repo",):
    if _p not in sys.path:
        sys.path.append(_p)

import numpy as np
import ml_dtypes

import concourse.bass as bass
import concourse.tile as tile
from concourse import bacc, mybir
from concourse.bass_utils import run_bass_kernel_spmd

NCORES = 8
D = 512
IMG = 96
N = IMG * IMG            # 9216 tokens
PIX = N // NCORES        # 1152 pixels per core (12 rows)
HSH = IMG // NCORES      # 12
AGPIX = 288              # 6 rows * 48
F32 = mybir.dt.float32
BF16 = mybir.dt.bfloat16
NPBF16 = ml_dtypes.bfloat16

_NCA = None
_NCB = None
LAST_RESULTS = None


def _build_a():
    """Per-core fold: (x, ag) shard -> S_loc (1, 512) f32."""
    nc = bacc.Bacc("TRN2", target_bir_lowering=False, debug=False,
                   num_devices=NCORES)

    xs = nc.declare_dram_parameter("xs", [128, 4608], BF16, isOutput=False)
    ags = nc.declare_dram_parameter("ags", [128, 2 * AGPIX], F32,
                                    isOutput=False)
    u1p = nc.declare_dram_parameter("u1p", [128, 4], BF16, isOutput=False)
    vvp = nc.declare_dram_parameter("vvp", [128, 8], F32, isOutput=False)
    sout = nc.declare_dram_parameter("sout", [1, D], F32, isOutput=True)

    with tile.TileContext(nc) as tc:
        with (
            tc.tile_pool(name="sb", bufs=1) as sb,
            tc.tile_pool(name="ps", bufs=1, space="PSUM") as ps,
        ):
            # tiny weights first so they don't queue behind the bulk x
            u1s = sb.tile([128, 4], BF16)
            nc.sync.dma_start(u1s[:], u1p[:, :])
            vvs = sb.tile([128, 8], F32)
            nc.scalar.dma_start(vvs[:], vvp[:, :])
            agc = sb.tile([128, 2 * AGPIX], F32)
            nc.gpsimd.dma_start(agc[:], ags[:, :])
            AG = [agc[:, 0:AGPIX], agc[:, AGPIX:2 * AGPIX]]

            # bulk x: 3 column chunks of [128, 1536] bf16 on the 3 queues
            xsb = sb.tile([128, 4608], BF16)
            xengs = (nc.sync, nc.scalar, nc.gpsimd)
            for c in range(3):
                xengs[c].dma_start(xsb[:, 1536 * c:1536 * (c + 1)],
                                   xs[:, 1536 * c:1536 * (c + 1)])

            # PE clock warm-up while data streams in
            wu = ps.tile([1, 4], F32)
            for _ in range(2):
                nc.tensor.matmul(wu[:1, :], u1s[:, 0:1], u1s[:, 0:4],
                                 start=True, stop=True)

            # --- ag path (f32, overlaps the x stream) ---
            AD = ps.tile([4, AGPIX], F32)
            for k in range(2):
                nc.tensor.matmul(AD[:, :], vvs[:, 4 * k:4 * k + 4], AG[k],
                                 start=(k == 0), stop=(k == 1))
            adsb = sb.tile([4, AGPIX], F32)
            nc.vector.tensor_copy(adsb[:, :], AD[:, :])
            tba = sb.tile([1, PIX], F32)
            nc.gpsimd.dma_start(tba[:1, :], adsb[:, :])
            tbp = sb.tile([1, PIX], F32)
            tb3 = tbp[:1, :].rearrange("p (y q) -> p y q", y=6)
            for a in range(2):
                for b in range(2):
                    st = 96 * a + b
                    dst = tb3[:, :, st:st + 95:2]              # (1, 6, 48)
                    s_ = tba[:1, 288 * (2 * a + b):288 * (2 * a + b) + 288
                             ].rearrange("p (y x) -> p y x", y=6)
                    nc.vector.tensor_copy(dst, s_)
            tbf = sb.tile([1, D], F32)
            nc.vector.tensor_add(tbf[:1, :], tbp[:1, 0:512], tbp[:1, 512:1024])
            nc.vector.tensor_add(tbf[:1, 0:128], tbf[:1, 0:128],
                                 tbp[:1, 1024:1152])

            # --- x fold: 12 matmuls, mod-512 fold + channel reduce in PSUM ---
            S = ps.tile([1, D], F32)
            S2 = ps.tile([1, 128], F32)
            for k in range(4):
                base = 1152 * k
                nc.tensor.matmul(S[:1, :], u1s[:, k:k + 1],
                                 xsb[:, base:base + 512],
                                 start=(k == 0), stop=False)
                nc.tensor.matmul(S[:1, :], u1s[:, k:k + 1],
                                 xsb[:, base + 512:base + 1024],
                                 start=False, stop=(k == 3))
                nc.tensor.matmul(S2[:1, :], u1s[:, k:k + 1],
                                 xsb[:, base + 1024:base + 1152],
                                 start=(k == 0), stop=(k == 3))

            srow = sb.tile([1, D], F32)
            nc.vector.tensor_add(srow[:1, :], S[:1, :], tbf[:1, :])
            nc.vector.tensor_add(srow[:1, 0:128], srow[:1, 0:128], S2[:1, :])
            nc.sync.dma_start(sout[:, :], srow[:1, :])

    nc.compile()
    return nc


def _build_b():
    """Per-core projection + broadcast write: s_glob -> out shard."""
    nc = bacc.Bacc("TRN2", target_bir_lowering=False, debug=False,
                   num_devices=NCORES)

    # cols 0:8 = s_glob transposed ([128,4] bf16 + 4 pad), 8:2056 = wV.T
    wvp = nc.declare_dram_parameter("wvp", [128, 2056], BF16, isOutput=False)
    out = nc.declare_dram_parameter("out", [128, 4608], F32, isOutput=True)

    with tile.TileContext(nc) as tc:
        with (
            tc.tile_pool(name="sb", bufs=1) as sb,
            tc.tile_pool(name="ps", bufs=1, space="PSUM") as ps,
        ):
            wvs = sb.tile([128, 2056], BF16)
            engs = (nc.sync, nc.scalar, nc.gpsimd)
            CH = ((0, 688), (688, 684), (1372, 684))
            for e, (c0, ln) in enumerate(CH):
                engs[e].dma_start(wvs[:, c0:c0 + ln], wvp[:, c0:c0 + ln])
            wu = ps.tile([1, 4], F32)
            for _ in range(2):
                nc.tensor.matmul(wu[:1, :], wvs[:, 0:1], wvs[:, 0:4],
                                 start=True, stop=True)

            # fused matvec + partition broadcast: stationary = s_glob column
            # replicated across all 128 M-columns -> bc[p, j] = s[j] for all p
            bc = ps.tile([128, D], F32)
            for k in range(4):
                lhsT = wvs[:, k:k + 1].to_broadcast((128, 128))
                nc.tensor.matmul(bc[:, :], lhsT,
                                 wvs[:, 8 + 512 * k:8 + 512 * (k + 1)],
                                 start=(k == 0), stop=(k == 3))
            bss = sb.tile([128, D], F32)
            nc.vector.tensor_copy(bss[:, :], bc[:, :])

            # window W = [s | s | s[:128]] then 4 column-block writes
            W = sb.tile([128, PIX], F32)
            nc.vector.tensor_copy(W[:, 0:512], bss[:, :])
            nc.vector.tensor_copy(W[:, 512:1024], bss[:, :])
            nc.vector.tensor_copy(W[:, 1024:1152], bss[:, 0:128])
            wengs = (nc.sync, nc.scalar, nc.gpsimd, nc.sync)
            for k in range(4):
                wengs[k].dma_start(out[:, 1152 * k:1152 * (k + 1)], W[:, :])

    nc.compile()
    return nc


def _get_ncs():
    global _NCA, _NCB
    if _NCA is None:
        _NCA = _build_a()
        _NCB = _build_b()
    return _NCA, _NCB


def _pe_colsum():
    pos = np.arange(N, dtype=np.float64)
    msk = np.arange(D)
    cos_msk = 1.0 - (msk % 2).astype(np.float64)
    freqs = (1e-4) ** ((2 * (msk // 2)).astype(np.float64) / D)
    ang = pos[:, None] * freqs[None, :]
    return (np.cos(ang) * cos_msk + np.sin(ang) * (1.0 - cos_msk)).sum(axis=0)


def kernel(x, y, ag, w_up, w_kv, w_q1, wQ, wK, wV):
    global LAST_RESULTS
    x = np.ascontiguousarray(x, dtype=np.float32)
    ag = np.ascontiguousarray(ag, dtype=np.float32)
    w_up = np.asarray(w_up, dtype=np.float32)
    w_kv = np.asarray(w_kv, dtype=np.float32)
    wV = np.ascontiguousarray(wV, dtype=np.float32)

    # ---- fold weights (host, ~1 MFLOP) ----
    u = w_kv.sum(axis=0)                      # (1024,)
    u1, u2 = u[:D], u[D:]
    v = np.einsum('iokw,o->ikw', w_up, u2)    # (256, 2, 2)

    u1p = np.ascontiguousarray(u1.reshape(4, 128).T).astype(NPBF16)
    vv = v.reshape(256, 4)
    vvp = np.ascontiguousarray(
        vv.reshape(2, 128, 4).transpose(1, 0, 2).reshape(128, 8))

    x2 = x.reshape(D, IMG, IMG)
    ag2 = ag.reshape(256, 48, 48)

    nca, ncb = _get_ncs()

    in_maps_a = []
    for i in range(NCORES):
        xsh = x2[:, HSH * i:HSH * (i + 1), :].reshape(D, PIX)
        xs = np.ascontiguousarray(
            xsh.reshape(4, 128, PIX).transpose(1, 0, 2).reshape(128, 4608)
        ).astype(NPBF16)
        in_maps_a.append({
            "xs": xs,
            "ags": np.ascontiguousarray(
                ag2[:, 6 * i:6 * (i + 1), :].reshape(2, 128, AGPIX)
                .transpose(1, 0, 2).reshape(128, 2 * AGPIX)),
            "u1p": u1p, "vvp": vvp,
        })
    res_a = run_bass_kernel_spmd(nca, in_maps_a, list(range(NCORES)))

    # ---- host gather: rotate + sum the 512-float partials, add pe ----
    s_glob = _pe_colsum()
    for i in range(NCORES):
        s_loc = res_a.results[i]["sout"].reshape(D).astype(np.float64)
        # local group t holds global group (t + 128*(i%4)) mod 512
        s_glob += np.roll(s_loc, 128 * (i % 4))
    s_glob = s_glob.astype(np.float32)
    sgp = np.zeros((128, 8), np.float32)
    sgp[:, 0:4] = s_glob.reshape(4, 128).T

    in_maps_b = []
    for i in range(NCORES):
        wvt = np.roll(wV, -128 * (i % 4), axis=0).T          # (512g, 512j)
        wvpa = np.empty((128, 2056), NPBF16)
        wvpa[:, 0:8] = sgp.astype(NPBF16)
        wvpa[:, 8:2056] = (
            wvt.reshape(4, 128, 512).transpose(1, 0, 2).reshape(128, 2048)
        ).astype(NPBF16)
        in_maps_b.append({"wvp": wvpa})
    res_b = run_bass_kernel_spmd(ncb, in_maps_b, list(range(NCORES)))
    LAST_RESULTS = (res_a, res_b)

    out2 = np.empty((D, N), dtype=np.float32)
    for i in range(NCORES):
        sh = res_b.results[i]["out"]                         # (128, 4608)
        out2[:, PIX * i:PIX * (i + 1)] = (
            sh.reshape(128, 4, PIX).transpose(1, 0, 2).reshape(D, PIX))
    return out2.reshape(1, D, IMG, IMG)
